# revision 1
# baseline (speedup 1.0000x reference)
"""Trainium2 (8-core) kernel for a GQA attention layer with rotary embeddings.

Reference computation (N=2048 tokens, D=1024, H=16 q-heads, KV=4 kv-heads, HD=64):
    xq = rope(x @ Wq), xk = rope(x @ Wk), xv = x @ Wv
    out = softmax(xq xk^T / sqrt(HD)) @ xv   (full attention, GQA)
    return out @ Wo

Sharding: heads across the 8 cores (2 q-heads + their shared kv-head per core),
attention computed per-core in a transposed (S^T) layout so no on-chip P
transpose is needed; softmax exp is split between the Scalar engine (table exp)
and a custom fused DVE op (cubic^4 approximation, valid because scores are
range-bounded); the per-head attention outputs are exchanged with a single
AllToAll so each core finishes the output projection for its own 256-token
slice with no core-dependent program constants.
"""

import numpy as np
import ml_dtypes

import concourse.bass as bass
import concourse.tile as tile
from concourse import bacc, mybir
from concourse.bass_utils import run_bass_kernel_spmd
from concourse.masks import make_identity

bf16 = ml_dtypes.bfloat16
BF16 = mybir.dt.bfloat16
F32 = mybir.dt.float32

N, D, H, KV, HD = 2048, 1024, 16, 4, 64
NC = 8
HPC = H // NC            # q-heads per core = 2
TOK = N // NC            # output token slice per core = 256
QW = HPC * N             # q free-axis width per core (heads concatenated) = 4096
NKB = N // 128           # 16 k-blocks of 128 tokens
VW = 128                 # [V (64 cols) | ones (64 cols)] per k-block: the ones
                         # columns make the PV matmul emit 64 identical sum rows,
                         # a free partition-broadcast for the normalize step

# exp(4y) ~= (1 + c1 y + c2 y^2 + c3 y^3)^4 minimax-fit on y in [-0.7, 0.7];
# scores/32 land in [-0.65, 0.65]. Max relative error ~0.65%.
EXP_C1, EXP_C2, EXP_C3 = 1.00305985, 0.51686418, 0.16136205

# which kbp iterations (per qb) run exp on the DVE instead of ACT
DVE_KBP = (1, 4, 6)


# ---------------------------------------------------------------- custom DVE op
def _register_exp_op():
    import concourse.dve_ops as dve_ops_mod
    from concourse.dve_spec import Spec, Src0, C0, C1, C2, One, sq, lower
    from concourse.dve_uop import DveOpSpec

    name = "EXP_CUBIC_POW4_ANT"
    for op in dve_ops_mod.OPS:
        if op.name == name:
            return op

    y = Src0
    p = ((C2 * y + C1) * y + C0) * y + One
    body = sq(sq(p))

    def ref(in0, in1, s0, s1, imm2):
        pp = ((imm2 * in0 + s1) * in0 + s0) * in0 + 1.0
        return (pp * pp) * (pp * pp)

    spec = Spec(body=body, reference=ref)
    row = dve_ops_mod._CUSTOM_DVE_ROW_BASE + len(dve_ops_mod.OPS)
    shas = {}
    for ver in ("v3", "v4"):
        try:
            uops = lower(spec, ver=ver)
            shas[ver] = DveOpSpec(name=name, opcode=row, uops=uops, rd1_en=False).sha(
                ver
            )
        except Exception:
            pass
    op = dve_ops_mod.DveOp(name, spec, subdim=False, uops_sha=shas)
    dve_ops_mod.OPS.append(op)
    dve_ops_mod.CUSTOM_DVE_SPECS[name] = spec
    dve_ops_mod._SUB_OPCODE_FOR_NAME[name] = row
    return op


EXP_OP = _register_exp_op()


# ---------------------------------------------------------------- device kernel
def _build_nc(dbg=False):
    nc = bacc.Bacc(
        "TRN2", target_bir_lowering=False, debug=False, num_devices=NC
    )
    xt = nc.dram_tensor("xt", [D // 128, 128, N], BF16, kind="ExternalInput").ap()
    wq = nc.dram_tensor("wq", [128, D // 128, 128], BF16, kind="ExternalInput").ap()
    wkv = nc.dram_tensor("wkv", [128, D // 128, 128], BF16, kind="ExternalInput").ap()
    wo = nc.dram_tensor("wo", [128, D // 128, D], BF16, kind="ExternalInput").ap()
    cosf = nc.dram_tensor("cosf", [128, N], BF16, kind="ExternalInput").ap()
    sinf = nc.dram_tensor("sinf", [128, N], BF16, kind="ExternalInput").ap()
    out = nc.dram_tensor("out", [TOK, D], F32, kind="ExternalOutput").ap()
    dbg_aps = None
    if dbg:
        dbg_aps = {
            name: nc.dram_tensor(f"dbg_{name}", shape, BF16, kind="ExternalOutput").ap()
            for name, shape in [
                ("qtd", [128, QW]), ("ktd", [128, N]), ("vp", [128, NKB * VW]),
                ("ofin", [128, N]), ("a2a", [NC, 128, TOK]), ("pt0", [128, 1024]),
                ("pt1", [128, 1024]),
            ]
        }

    with tile.TileContext(nc) as tc:
        _emit(nc, tc, xt, wq, wkv, wo, cosf, sinf, out, dbg_aps)
    nc.compile()
    return nc


def _emit(nc, tc, xt, wq, wkv, wo, cosf, sinf, out, dbg_aps=None):
    fexp = mybir.ActivationFunctionType.Exp
    NCHUNK = D // 128  # 8 contraction chunks for the projections

    with (
        tc.tile_pool(name="persist", bufs=1) as pp,
        tc.tile_pool(name="work", bufs=3) as wp,
        tc.tile_pool(name="ppool", bufs=4) as ppool,
        tc.tile_pool(name="dram", bufs=1, space="DRAM") as dram,
    ):
        # ---- persistent SBUF tensors
        xall = pp.tile([128, NCHUNK, N], BF16, tag="xall")
        wq_sb = pp.tile([128, NCHUNK, 128], BF16, tag="wq")
        wkv_sb = pp.tile([128, NCHUNK, 128], BF16, tag="wkv")
        wo_sb = pp.tile([128, NCHUNK, D], BF16, tag="wo")
        cos_sb = pp.tile([128, N], BF16, tag="cos")
        sin_sb = pp.tile([128, N], BF16, tag="sin")
        qtd = pp.tile([128, QW], BF16, tag="qtd")       # q^T, both heads on free axis, rows duplicated
        ktd = pp.tile([128, N], BF16, tag="ktd")        # k^T duplicated rows
        vp = pp.tile([128, NKB * VW], BF16, tag="vp")   # [V | ones] per k-block
        ofin = pp.tile([128, N], BF16, tag="ofin")      # normalized attention out^T
        ident = pp.tile([128, 128], BF16, tag="ident")

        for c in range(NCHUNK):
            nc.sync.dma_start(xall[:, c, :], xt[c])
        nc.sync.dma_start(wq_sb[:], wq)
        nc.sync.dma_start(wkv_sb[:], wkv)
        nc.sync.dma_start(wo_sb[:], wo)
        nc.sync.dma_start(cos_sb[:], cosf)
        nc.sync.dma_start(sin_sb[:], sinf)
        make_identity(nc, ident[:])
        nc.vector.memset(vp[:], 1.0)

        # ---- projections + rope + V transpose
        with (
            tc.tile_pool(name="proj_ps", bufs=2, space="PSUM") as proj_ps,
            tc.tile_pool(name="vt_ps", bufs=2, space="PSUM") as vt_ps,
        ):
            for tb in range(4):  # token blocks of 512
                ts_ = slice(tb * 512, (tb + 1) * 512)
                pq = proj_ps.tile([128, 512], F32, tag="pq")
                for c in range(NCHUNK):
                    nc.tensor.matmul(
                        pq[:], wq_sb[:, c, :], xall[:, c, ts_],
                        start=(c == 0), stop=(c == NCHUNK - 1),
                    )
                pkv = proj_ps.tile([128, 512], F32, tag="pkv")
                for c in range(NCHUNK):
                    nc.tensor.matmul(
                        pkv[:], wkv_sb[:, c, :], xall[:, c, ts_],
                        start=(c == 0), stop=(c == NCHUNK - 1),
                    )

                qsb = wp.tile([128, 512], BF16, tag="qsb")
                nc.scalar.copy(qsb[:], pq[:])
                kvsb = wp.tile([128, 512], BF16, tag="kvsb")
                nc.scalar.copy(kvsb[:], pkv[:])

                # rope as q' = q*cos + swap(q)*[-s;+s], with the half-swap done
                # by DMA so every DVE op is partition-aligned
                qsw = wp.tile([128, 512], BF16, tag="qsw")
                for b in (0, 64):
                    nc.sync.dma_start(qsw[b : b + 32, :], qsb[b + 32 : b + 64, :])
                    nc.sync.dma_start(qsw[b + 32 : b + 64, :], qsb[b : b + 32, :])
                t1 = wp.tile([128, 512], BF16, tag="ropet1")
                nc.vector.tensor_mul(t1[:], qsb[:], cos_sb[:, ts_])
                t2 = wp.tile([128, 512], BF16, tag="ropet2")
                nc.vector.tensor_mul(t2[:], qsw[:], sin_sb[:, ts_])
                qr = wp.tile([128, 512], BF16, tag="qrot")
                nc.vector.tensor_add(qr[:], t1[:], t2[:])
                for h in range(HPC):
                    dst = slice(h * N + tb * 512, h * N + (tb + 1) * 512)
                    nc.sync.dma_start(qtd[0:64, dst], qr[64 * h : 64 * h + 64, :])
                    nc.sync.dma_start(qtd[64:128, dst], qr[64 * h : 64 * h + 64, :])

                # rope for k (kv rows 0-63), same scheme
                ksw = wp.tile([64, 512], BF16, tag="ksw")
                nc.sync.dma_start(ksw[0:32, :], kvsb[32:64, :])
                nc.sync.dma_start(ksw[32:64, :], kvsb[0:32, :])
                t1k = wp.tile([64, 512], BF16, tag="ropet1k")
                nc.vector.tensor_mul(t1k[:], kvsb[0:64, :], cos_sb[0:64, ts_])
                t2k = wp.tile([64, 512], BF16, tag="ropet2k")
                nc.vector.tensor_mul(t2k[:], ksw[:], sin_sb[0:64, ts_])
                kr = wp.tile([64, 512], BF16, tag="krot")
                nc.vector.tensor_add(kr[:], t1k[:], t2k[:])
                nc.sync.dma_start(ktd[0:64, ts_], kr[:])
                nc.sync.dma_start(ktd[64:128, ts_], kr[:])

                # V natural: transpose v^T (rows 64-127 of kvsb) in 128-col blocks
                for j in range(4):
                    kb = tb * 4 + j
                    vt = vt_ps.tile([128, 64], BF16, tag="vt")
                    nc.tensor.transpose(
                        vt[:],
                        kvsb[64:128, j * 128 : (j + 1) * 128],
                        ident[64:128, 64:128],
                    )
                    nc.vector.tensor_copy(
                        vp[:, kb * VW + 64 : kb * VW + 64 + HD], vt[:]
                    )


        # ---- attention: S^T = k q^T per k-block, exp, PV accumulate
        with (
            tc.tile_pool(name="stage_ps", bufs=2, space="PSUM") as stage_ps,
            tc.tile_pool(name="acc_ps", bufs=2, space="PSUM") as acc_ps,
        ):
            for qb in range(QW // 512):  # 8 blocks of 512 q-columns
                qs = slice(qb * 512, (qb + 1) * 512)
                acc = acc_ps.tile([128, 512], F32, tag="acc")
                for kbp in range(NKB // 2):
                    kb0, kb1 = 2 * kbp, 2 * kbp + 1
                    st = stage_ps.tile([128, 1024], F32, tag="stage")
                    nc.tensor.matmul(
                        st[:, 0:512],
                        ktd[0:64, kb0 * 128 : (kb0 + 1) * 128],
                        qtd[0:64, qs],
                        start=True, stop=True,
                    )
                    nc.tensor.matmul(
                        st[:, 512:1024],
                        ktd[64:128, kb1 * 128 : (kb1 + 1) * 128],
                        qtd[64:128, qs],
                        start=True, stop=True,
                    )
                    pt = ppool.tile([128, 1024], BF16, tag="pt")
                    if kbp in DVE_KBP:
                        nc.vector._custom_dve(
                            EXP_OP, out=pt[:], in0=st[:],
                            s0=EXP_C1, s1=EXP_C2, imm2=EXP_C3,
                        )
                    else:
                        nc.scalar.activation(pt[:], st[:], fexp, scale=4.0)
                    if dbg_aps is not None and qb == 0 and kbp == 0:
                        nc.sync.dma_start(dbg_aps["pt0"], pt[:])
                    if dbg_aps is not None and qb == 0 and kbp == 1:
                        nc.sync.dma_start(dbg_aps["pt1"], pt[:])
                    nc.tensor.matmul(
                        acc[:],
                        vp[:, kb0 * VW : (kb0 + 1) * VW],
                        pt[:, 0:512],
                        start=(kbp == 0), stop=False,
                    )
                    nc.tensor.matmul(
                        acc[:],
                        vp[:, kb1 * VW : (kb1 + 1) * VW],
                        pt[:, 512:1024],
                        start=False, stop=(kbp == NKB // 2 - 1),
                    )
                # normalize: copy acc to SBUF, realign the out-half to
                # partition base 0 by DMA, then all ops are base-aligned
                asb = wp.tile([128, 512], F32, tag="asb")
                nc.scalar.copy(asb[:], acc[:])
                obuf = wp.tile([64, 512], F32, tag="obuf")
                nc.sync.dma_start(obuf[:], asb[64:128, :])
                rs = wp.tile([64, 512], F32, tag="rsum")
                nc.vector.reciprocal_approx_fast(rs[:], asb[0:64, :])
                ot = wp.tile([64, 512], BF16, tag="onorm")
                nc.vector.tensor_mul(ot[:], obuf[:], rs[:])
                h = qb // 4
                toks = slice((qb % 4) * 512, (qb % 4 + 1) * 512)
                nc.sync.dma_start(ofin[64 * h : 64 * h + 64, toks], ot[:])

        if dbg_aps is not None:
            nc.sync.dma_start(dbg_aps["qtd"], qtd[:])
            nc.sync.dma_start(dbg_aps["ktd"], ktd[:])
            nc.sync.dma_start(dbg_aps["vp"], vp[:])
            nc.sync.dma_start(dbg_aps["ofin"], ofin[:])

        # ---- AllToAll: send token-chunk j of our heads to core j
        a2a_in = dram.tile([NC, 128, TOK], BF16)
        a2a_out = dram.tile([NC, 128, TOK], BF16)
        for j in range(NC):
            nc.sync.dma_start(a2a_in[j], ofin[:, j * TOK : (j + 1) * TOK])
        nc.gpsimd.collective_compute(
            "AllToAll",
            mybir.AluOpType.bypass,
            replica_groups=[list(range(NC))],
            ins=[a2a_in.opt()],
            outs=[a2a_out.opt()],
        )

        if dbg_aps is not None:
            # bounce a2a_out through SBUF (DMA dram->dram is fine too, keep simple)
            nc.sync.dma_start(dbg_aps["a2a"], a2a_out[:])

        # ---- output projection for our 256-token slice
        og = pp.tile([128, NCHUNK, TOK], BF16, tag="og")
        nc.sync.dma_start(og[:], a2a_out.rearrange("c p t -> p c t"))
        with tc.tile_pool(name="oproj_ps", bufs=4, space="PSUM") as ops_:
            for m in range(2):
                for n_ in range(2):
                    po = ops_.tile([128, 512], F32, tag="po")
                    for c in range(NCHUNK):
                        nc.tensor.matmul(
                            po[:],
                            og[:, c, m * 128 : (m + 1) * 128],
                            wo_sb[:, c, n_ * 512 : (n_ + 1) * 512],
                            start=(c == 0), stop=(c == NCHUNK - 1),
                        )
                    osb = wp.tile([128, 512], F32, tag="osb")
                    nc.scalar.copy(osb[:], po[:])
                    nc.sync.dma_start(
                        out[m * 128 : (m + 1) * 128, n_ * 512 : (n_ + 1) * 512],
                        osb[:],
                    )


_NC_CACHE = None


def _get_nc():
    global _NC_CACHE
    if _NC_CACHE is None:
        _NC_CACHE = _build_nc()
    return _NC_CACHE


# ---------------------------------------------------------------- host wrapper
_ROPE_PERM = np.concatenate([np.arange(0, HD, 2), np.arange(1, HD, 2)])


def _chunked(w):
    """(D, F) -> (128, D//128, F) so [p, c, f] = w[128c+p, f]."""
    return np.ascontiguousarray(
        w.reshape(D // 128, 128, -1).transpose(1, 0, 2)
    )


def _prep_inputs(x, freqs_cos, freqs_sin, Wq, Wk, Wv, Wo):
    x = np.asarray(x, np.float32)
    Wq = np.asarray(Wq, np.float32)
    Wk = np.asarray(Wk, np.float32)
    Wv = np.asarray(Wv, np.float32)
    Wo = np.asarray(Wo, np.float32)
    cos = np.asarray(freqs_cos, np.float32)
    sin = np.asarray(freqs_sin, np.float32)

    xt = np.ascontiguousarray(x.T).reshape(D // 128, 128, N).astype(bf16)
    cosf = np.tile(cos.T, (4, 1)).astype(bf16)
    # signed sin table matching the [real(32); imag(32)] row blocks:
    # q' = q*cos + swap(q)*[-s; +s]
    sinf = np.tile(np.concatenate([-sin.T, sin.T], axis=0), (2, 1)).astype(bf16)
    wo_dev = _chunked(Wo).astype(bf16)

    in_maps = []
    for r in range(NC):
        h0, h1 = 2 * r, 2 * r + 1
        g = r // 2
        # q pre-scaled by 1/32: folds the 1/sqrt(HD)=1/8 softmax scale and the
        # /4 for the (cubic)^4 exp decomposition into the weights.
        wq_core = np.concatenate(
            [
                Wq[:, 64 * h0 + _ROPE_PERM],
                Wq[:, 64 * h1 + _ROPE_PERM],
            ],
            axis=1,
        ) * (1.0 / 32.0)
        wkv_core = np.concatenate(
            [Wk[:, 64 * g + _ROPE_PERM], Wv[:, 64 * g : 64 * g + HD]], axis=1
        )
        in_maps.append(
            {
                "xt": xt,
                "wq": _chunked(wq_core).astype(bf16),
                "wkv": _chunked(wkv_core).astype(bf16),
                "wo": wo_dev,
                "cosf": cosf,
                "sinf": sinf,
            }
        )
    return in_maps


def _run(inputs, trace=False, dbg=False, **spmd_kwargs):
    in_maps = _prep_inputs(**inputs)
    nc = _build_nc(dbg=True) if dbg else _get_nc()
    res = run_bass_kernel_spmd(
        nc, in_maps, core_ids=list(range(NC)), trace=trace, **spmd_kwargs
    )
    full = np.concatenate([res.results[r]["out"] for r in range(NC)], axis=0)
    return full.astype(np.float32), res


def kernel(**inputs):
    out, _ = _run(inputs, trace=False)
    return out



# revision 2
# speedup vs baseline: 1.1192x; 1.1192x over previous
"""Trainium2 (8-core) kernel for a GQA attention layer with rotary embeddings.

Reference computation (N=2048 tokens, D=1024, H=16 q-heads, KV=4 kv-heads, HD=64):
    xq = rope(x @ Wq), xk = rope(x @ Wk), xv = x @ Wv
    out = softmax(xq xk^T / sqrt(HD)) @ xv   (full attention, GQA)
    return out @ Wo

Sharding: 2 q-heads + their shared kv-head per core. Attention runs in a
transposed (S^T) layout, token-block-major, with the two q-heads row-packed
into one concurrent PE pair per k-block. Softmax exp is split between the
Scalar engine (table exp) and a custom DVE op (cubic^4 approximation). The
rope half-swap is done with DVE stream_shuffle (no DMA round trips). Each
512-token block's head outputs are exchanged with a small AllToAll as soon
as they finish (4 collectives, interleaved destinations), so the output
projection overlaps the attention tail.
"""

import numpy as np
import ml_dtypes

import concourse.bass as bass
import concourse.tile as tile
from concourse import bacc, mybir
from concourse.bass_utils import run_bass_kernel_spmd
from concourse.masks import make_identity

bf16 = ml_dtypes.bfloat16
BF16 = mybir.dt.bfloat16
F32 = mybir.dt.float32

N, D, H, KV, HD = 2048, 1024, 16, 4, 64
NC = 8
HPC = H // NC            # q-heads per core = 2
NTB = 4                  # token blocks of 512
TBW = N // NTB           # 512
NKB = N // 128           # 16 k-blocks of 128 tokens
NCHUNK = D // 128        # 8 contraction chunks

# exp(4y) ~= (1 + c1 y + c2 y^2 + c3 y^3)^4 minimax-fit on y in [-0.7, 0.7];
# scores/32 land in [-0.65, 0.65]. Max relative error ~0.65%.
EXP_C1, EXP_C2, EXP_C3 = 1.00305985, 0.51686418, 0.16136205

# k-blocks whose exp runs on the DVE (custom op); the rest go to ACT.
DVE_KB = frozenset({1, 3, 5, 7, 9, 11, 13})

IDENT32 = list(range(32))


# ---------------------------------------------------------------- custom DVE op
def _register_exp_op():
    import concourse.dve_ops as dve_ops_mod
    from concourse.dve_spec import Spec, Src0, C0, C1, C2, One, sq, lower
    from concourse.dve_uop import DveOpSpec

    name = "EXP_CUBIC_POW4_ANT"
    for op in dve_ops_mod.OPS:
        if op.name == name:
            return op

    y = Src0
    p = ((C2 * y + C1) * y + C0) * y + One
    body = sq(sq(p))

    def ref(in0, in1, s0, s1, imm2):
        pp = ((imm2 * in0 + s1) * in0 + s0) * in0 + 1.0
        return (pp * pp) * (pp * pp)

    spec = Spec(body=body, reference=ref)
    row = dve_ops_mod._CUSTOM_DVE_ROW_BASE + len(dve_ops_mod.OPS)
    shas = {}
    for ver in ("v3", "v4"):
        try:
            uops = lower(spec, ver=ver)
            shas[ver] = DveOpSpec(name=name, opcode=row, uops=uops, rd1_en=False).sha(
                ver
            )
        except Exception:
            pass
    op = dve_ops_mod.DveOp(name, spec, subdim=False, uops_sha=shas)
    dve_ops_mod.OPS.append(op)
    dve_ops_mod.CUSTOM_DVE_SPECS[name] = spec
    dve_ops_mod._SUB_OPCODE_FOR_NAME[name] = row
    return op


EXP_OP = _register_exp_op()


# ---------------------------------------------------------------- device kernel
def _build_nc(dbg=False):
    nc = bacc.Bacc(
        "TRN2", target_bir_lowering=False, debug=False, num_devices=NC
    )
    xt = nc.dram_tensor("xt", [128, NTB, NCHUNK, TBW], BF16, kind="ExternalInput").ap()
    wq = nc.dram_tensor("wq", [128, NCHUNK, 128], BF16, kind="ExternalInput").ap()
    wkv = nc.dram_tensor("wkv", [128, NCHUNK, 128], BF16, kind="ExternalInput").ap()
    wo = nc.dram_tensor("wo", [128, NCHUNK, D], BF16, kind="ExternalInput").ap()
    cosf = nc.dram_tensor("cosf", [128, N], BF16, kind="ExternalInput").ap()
    sinf = nc.dram_tensor("sinf", [128, N], BF16, kind="ExternalInput").ap()
    out = nc.dram_tensor("out", [NTB, 64, D], F32, kind="ExternalOutput").ap()
    dbg_aps = None
    if dbg:
        dbg_aps = {
            name: nc.dram_tensor(f"dbg_{name}", shape, BF16, kind="ExternalOutput").ap()
            for name, shape in [
                ("qtd", [128, N]), ("ktd", [128, N]), ("vp", [128, NKB * 128]),
                ("ofin", [128, N]), ("og", [128, NCHUNK, NTB, 64]),
            ]
        }

    with tile.TileContext(nc) as tc:
        _emit(nc, tc, xt, wq, wkv, wo, cosf, sinf, out, dbg_aps)
    nc.compile()
    return nc


def _emit(nc, tc, xt, wq, wkv, wo, cosf, sinf, out, dbg_aps=None):
    fexp = mybir.ActivationFunctionType.Exp

    with (
        tc.tile_pool(name="persist", bufs=1) as pp,
        tc.tile_pool(name="work", bufs=3) as wp,
        tc.tile_pool(name="big_ps", bufs=3, space="PSUM") as big_ps,
        tc.tile_pool(name="u_ps", bufs=2, space="PSUM") as u_ps,
        tc.tile_pool(name="ptp", bufs=4) as ptp,
        tc.tile_pool(name="dram", bufs=1, space="DRAM") as dram,
    ):
        # ---- persistent SBUF tensors
        xall = pp.tile([128, NTB, NCHUNK, TBW], BF16, tag="xall")
        wq_sb = pp.tile([128, NCHUNK, 128], BF16, tag="wq")
        wkv_sb = pp.tile([128, NCHUNK, 128], BF16, tag="wkv")
        wo_sb = pp.tile([128, NCHUNK, D], BF16, tag="wo")
        cos_sb = pp.tile([128, N], BF16, tag="cos")
        sin_sb = pp.tile([128, N], BF16, tag="sin")
        qtd = pp.tile([128, N], BF16, tag="qtd")      # q^T: rows = [h0(r,i) | h1(r,i)]
        ktd = pp.tile([128, N], BF16, tag="ktd")      # k^T: rows 0:63 = k, 64:127 dup
        vp = pp.tile([128, NKB * 128], BF16, tag="vp")  # [ones(64) | V(64)] per k-block
        ofin = pp.tile([128, N], BF16, tag="ofin")    # normalized attention out^T
        og = pp.tile([128, NCHUNK, NTB, 64], BF16, tag="og")  # gathered o for oproj
        ident = pp.tile([128, 128], BF16, tag="ident")

        nc.sync.dma_start(wq_sb[:], wq)
        nc.sync.dma_start(wkv_sb[:], wkv)
        nc.sync.dma_start(cos_sb[:], cosf)
        nc.sync.dma_start(sin_sb[:], sinf)
        for tb in range(NTB):
            nc.sync.dma_start(xall[:, tb], xt[:, tb])
        nc.sync.dma_start(wo_sb[:], wo)
        make_identity(nc, ident[:])
        nc.vector.memset(vp[:], 1.0)

        # ---- projections + rope (stream_shuffle half-swap) + V transpose
        for tb in range(NTB):
            ts_ = slice(tb * TBW, (tb + 1) * TBW)
            pqkv = big_ps.tile([128, 1024], F32, tag="big")
            for c in range(NCHUNK):
                nc.tensor.matmul(
                    pqkv[:, 0:512], wq_sb[:, c, :], xall[:, tb, c, :],
                    start=(c == 0), stop=(c == NCHUNK - 1),
                )
            for c in range(NCHUNK):
                nc.tensor.matmul(
                    pqkv[:, 512:1024], wkv_sb[:, c, :], xall[:, tb, c, :],
                    start=(c == 0), stop=(c == NCHUNK - 1),
                )
            qk = wp.tile([128, 1024], BF16, tag="qksb")  # [q | kv]
            nc.scalar.copy(qk[:, 0:512], pqkv[:, 0:512])
            nc.scalar.copy(qk[:, 512:1024], pqkv[:, 512:1024])

            # rope: q' = q*cos + swap(q)*[-s;+s]; swap via DVE stream_shuffle
            qsw = wp.tile([128, 512], BF16, tag="qsw")
            nc.vector.stream_shuffle(qsw[0:32, :], qk[32:64, 0:512], IDENT32)
            nc.vector.stream_shuffle(qsw[32:64, :], qk[0:32, 0:512], IDENT32)
            nc.vector.stream_shuffle(qsw[64:96, :], qk[96:128, 0:512], IDENT32)
            nc.vector.stream_shuffle(qsw[96:128, :], qk[64:96, 0:512], IDENT32)
            t1 = wp.tile([128, 512], BF16, tag="ropet1")
            nc.vector.tensor_mul(t1[:], qk[:, 0:512], cos_sb[:, ts_])
            t2 = wp.tile([128, 512], BF16, tag="ropet2")
            nc.vector.tensor_mul(t2[:], qsw[:], sin_sb[:, ts_])
            nc.vector.tensor_add(qtd[:, ts_], t1[:], t2[:])

            ksw = wp.tile([64, 512], BF16, tag="ksw")
            nc.vector.stream_shuffle(ksw[0:32, :], qk[32:64, 512:1024], IDENT32)
            nc.vector.stream_shuffle(ksw[32:64, :], qk[0:32, 512:1024], IDENT32)
            t1k = wp.tile([64, 512], BF16, tag="ropet1k")
            nc.vector.tensor_mul(t1k[:], qk[0:64, 512:1024], cos_sb[0:64, ts_])
            t2k = wp.tile([64, 512], BF16, tag="ropet2k")
            nc.vector.tensor_mul(t2k[:], ksw[:], sin_sb[0:64, ts_])
            nc.vector.tensor_add(ktd[0:64, ts_], t1k[:], t2k[:])

            # V natural: transpose v^T (rows 64:127 of kv half) per 128-col block
            for j in range(4):
                kb = tb * 4 + j
                vt = u_ps.tile([128, 64], BF16, tag="u")
                nc.tensor.transpose(
                    vt[0:128, 0:64],
                    qk[64:128, 512 + j * 128: 512 + (j + 1) * 128],
                    ident[64:128, 64:128],
                )
                nc.vector.tensor_copy(vp[:, kb * 128 + 64: kb * 128 + 128], vt[:, 0:64])

        # duplicate k rows for the row-packed S pairs
        nc.sync.dma_start(ktd[64:128, :], ktd[0:64, :])

        # ---- attention + interleaved AllToAll + output projection
        a2a_outs = []
        for tb in range(NTB):
            ts_ = slice(tb * TBW, (tb + 1) * TBW)
            acc0 = u_ps.tile([128, 512], F32, tag="u")
            acc1 = u_ps.tile([128, 512], F32, tag="u")
            for kb in range(NKB):
                ks_ = slice(kb * 128, (kb + 1) * 128)
                st = big_ps.tile([128, 1024], F32, tag="big")
                nc.tensor.matmul(
                    st[:, 0:512], ktd[0:64, ks_], qtd[0:64, ts_],
                    start=True, stop=True,
                )
                nc.tensor.matmul(
                    st[:, 512:1024], ktd[64:128, ks_], qtd[64:128, ts_],
                    start=True, stop=True,
                )
                pt = ptp.tile([128, 1024], BF16, tag="pt")
                if kb in DVE_KB:
                    nc.vector._custom_dve(
                        EXP_OP, out=pt[:], in0=st[:],
                        s0=EXP_C1, s1=EXP_C2, imm2=EXP_C3,
                    )
                else:
                    nc.scalar.activation(pt[:], st[:], fexp, scale=4.0)
                nc.tensor.matmul(
                    acc0[:], vp[:, ks_], pt[:, 0:512],
                    start=(kb == 0), stop=(kb == NKB - 1),
                )
                nc.tensor.matmul(
                    acc1[:], vp[:, ks_], pt[:, 512:1024],
                    start=(kb == 0), stop=(kb == NKB - 1),
                )

            # normalize: sums on rows 0:63 (ones cols), V-out on rows 64:127
            for h, acc in ((0, acc0), (1, acc1)):
                asb = wp.tile([128, 512], F32, tag="asb")
                nc.scalar.copy(asb[:], acc[:])
                obuf = wp.tile([64, 512], F32, tag="obuf")
                nc.sync.dma_start(obuf[:], asb[64:128, :])
                rs = wp.tile([64, 512], F32, tag="rsum")
                nc.vector.reciprocal_approx_fast(rs[:], asb[0:64, :])
                if h == 0:
                    nc.vector.tensor_mul(ofin[0:64, ts_], obuf[:], rs[:])
                else:
                    ot = wp.tile([64, 512], BF16, tag="onorm")
                    nc.vector.tensor_mul(ot[:], obuf[:], rs[:])
                    nc.sync.dma_start(ofin[64:128, ts_], ot[:])

            # AllToAll for this token block: 64-token strip j -> core j
            a2a_in = dram.tile([NC, 128, 64], BF16, tag=f"a2ain{tb}")
            a2a_out = dram.tile([NC, 128, 64], BF16, tag=f"a2aout{tb}")
            nc.sync.dma_start(a2a_in.rearrange("j p t -> p j t"), ofin[:, ts_])
            nc.gpsimd.collective_compute(
                "AllToAll",
                mybir.AluOpType.bypass,
                replica_groups=[list(range(NC))],
                ins=[a2a_in.opt()],
                outs=[a2a_out.opt()],
            )
            a2a_outs.append(a2a_out)
            nc.sync.dma_start(og[:, :, tb, :], a2a_out.rearrange("c p t -> p c t"))

            # output projection for gathered halves, one pass per 2 blocks
            if tb in (2, 3):
                P = tb - 2
                po = big_ps.tile([128, 1024], F32, tag="big")
                for n_ in range(2):
                    for c in range(NCHUNK):
                        nc.tensor.matmul(
                            po[:, n_ * 512:(n_ + 1) * 512],
                            og[:, c, 2 * P: 2 * P + 2, :],
                            wo_sb[:, c, n_ * 512:(n_ + 1) * 512],
                            start=(c == 0), stop=(c == NCHUNK - 1),
                        )
                osb = wp.tile([128, 1024], F32, tag="osb")
                nc.scalar.copy(osb[:, 0:512], po[:, 0:512])
                nc.scalar.copy(osb[:, 512:1024], po[:, 512:1024])
                nc.sync.dma_start(
                    out[2 * P: 2 * P + 2].rearrange("b t d -> (b t) d"), osb[:]
                )

        if dbg_aps is not None:
            nc.sync.dma_start(dbg_aps["qtd"], qtd[:])
            nc.sync.dma_start(dbg_aps["ktd"], ktd[:])
            nc.sync.dma_start(dbg_aps["vp"], vp[:])
            nc.sync.dma_start(dbg_aps["ofin"], ofin[:])
            nc.sync.dma_start(dbg_aps["og"], og[:])


_NC_CACHE = None


def _get_nc():
    global _NC_CACHE
    if _NC_CACHE is None:
        _NC_CACHE = _build_nc()
    return _NC_CACHE


# ---------------------------------------------------------------- host wrapper
_ROPE_PERM = np.concatenate([np.arange(0, HD, 2), np.arange(1, HD, 2)])


def _chunked(w):
    """(D, F) -> (128, D//128, F) so [p, c, f] = w[128c+p, f]."""
    return np.ascontiguousarray(
        w.reshape(D // 128, 128, -1).transpose(1, 0, 2)
    )


def _prep_inputs(x, freqs_cos, freqs_sin, Wq, Wk, Wv, Wo):
    x = np.asarray(x, np.float32)
    Wq = np.asarray(Wq, np.float32)
    Wk = np.asarray(Wk, np.float32)
    Wv = np.asarray(Wv, np.float32)
    Wo = np.asarray(Wo, np.float32)
    cos = np.asarray(freqs_cos, np.float32)
    sin = np.asarray(freqs_sin, np.float32)

    # xt[p, tb, c, n] = x[512*tb + n, 128*c + p]
    xtv = np.ascontiguousarray(
        x.T.reshape(NCHUNK, 128, NTB, TBW).transpose(1, 2, 0, 3)
    ).astype(bf16)
    cosf = np.tile(cos.T, (4, 1)).astype(bf16)
    sinf = np.tile(np.concatenate([-sin.T, sin.T], axis=0), (2, 1)).astype(bf16)
    wo_dev = _chunked(Wo).astype(bf16)

    in_maps = []
    for r in range(NC):
        h0, h1 = 2 * r, 2 * r + 1
        g = r // 2
        # q pre-scaled by 1/32: folds the 1/sqrt(HD)=1/8 softmax scale and the
        # /4 for the (cubic)^4 exp decomposition into the weights.
        wq_core = np.concatenate(
            [
                Wq[:, 64 * h0 + _ROPE_PERM],
                Wq[:, 64 * h1 + _ROPE_PERM],
            ],
            axis=1,
        ) * (1.0 / 32.0)
        wkv_core = np.concatenate(
            [Wk[:, 64 * g + _ROPE_PERM], Wv[:, 64 * g: 64 * g + HD]], axis=1
        )
        in_maps.append(
            {
                "xt": xtv,
                "wq": _chunked(wq_core).astype(bf16),
                "wkv": _chunked(wkv_core).astype(bf16),
                "wo": wo_dev,
                "cosf": cosf,
                "sinf": sinf,
            }
        )
    return in_maps


def _run(inputs, trace=False, dbg=False, **spmd_kwargs):
    in_maps = _prep_inputs(**inputs)
    nc = _build_nc(dbg=True) if dbg else _get_nc()
    res = run_bass_kernel_spmd(
        nc, in_maps, core_ids=list(range(NC)), trace=trace, **spmd_kwargs
    )
    # core r computed tokens {512*b + 64*r + t} for b in 0..3, t in 0..63
    full = np.empty((N, D), np.float32)
    for r in range(NC):
        o = res.results[r]["out"].reshape(NTB, 64, D)
        for b in range(NTB):
            full[512 * b + 64 * r: 512 * b + 64 * r + 64] = o[b]
    return full, res


def kernel(**inputs):
    out, _ = _run(inputs, trace=False)
    return out
